# revision 4
# baseline (speedup 1.0000x reference)
"""MoE-attention kernel v2 for 8 Trainium2 NeuronCores.

Sharding: expert-parallel within sequence pairs. Cores (2b, 2b+1) both hold
sequence b; core 2b computes experts 0-9, core 2b+1 experts 10-19, each for
ALL 1024 tokens (QKV projections + attention have zero duplicated work; only
the tiny router runs twice). Each core owns the output for its token half;
the 640 feature columns it computed for the partner's half are exchanged via
a pairwise AllGather, then each core runs the full out_proj for its 512
tokens.

Uniform-program tricks (all core-dependence lives in host-prepared data):
  - token order per core: own half first (cols 0-511), other half second
  - expert order per core: own 10 experts first (router cols, wv cols)
  - out_w rows reordered: own experts' feature rows first, partner's second
  - AllGather slot select via per-core 0/1 mask columns

Schedule: V/router first, then per expert pair p: K/Q proj -> other-half
attention for its 2 experts -> gating -> eager fx_out chunk DMA. AllGather
overlaps the own-half attention pass; out_proj runs last. Scores are emitted
one key-tile-pair ahead of the eoT accumulation so the scalar engine's Exp
runs concurrently with PE. Reciprocals are batched per pair ([2,512]).

All matmul operands bf16 (f32 PSUM). Layouts:
  xT       [128, 10dt, 1024t]  host-pre-transposed input
  k/q_all  [128(pair), 5p, 1024t]
  v_sb     [128t, 8kt, 10e, 65]  natural V + ones column -> sumexp row
  scoresT  [128k, 1024q] psum (2 key tiles / psum tile, one Exp per tile)
  eoT      [65h, 512q] accumulated over 8 key tiles
  comb     [128h', 10ht, 512t]  own-half gated features (ht 0-4 local,
           5-9 partner via exchange); combX [128, 5, 512] other-half
  out      [512t, 1280] = comb.T @ ow + ob
"""

import numpy as np

import concourse.bass as bass
import concourse.mybir as mybir
import concourse.tile as tile
from concourse import bacc
from concourse.bass_utils import run_bass_kernel_spmd

F32 = mybir.dt.float32
BF16 = mybir.dt.bfloat16
try:
    import ml_dtypes as _mld
    NP_BF16 = np.dtype(_mld.bfloat16)
except Exception:  # pragma: no cover
    NP_BF16 = np.float32
AF = mybir.ActivationFunctionType

B = 4
S = 1024
D = 1280
E = 20
EL = 10           # experts per core
H = 64
SL = 512          # own-half query tokens per core
DT = D // 128     # 10
KT = S // 128     # 8
NP_ = EL // 2     # 5 own expert pairs
SCALE = float(H) ** -0.5
NCORES = 8
RWX = 68          # router columns incl. aligned group copies
OC_CHUNKS = [(0, 512), (512, 512), (1024, 256)]


def _attn_expert(nc, scp, atp, eop, k_all, q_all, v_sb, e, hsl):
    """Pipelined scoresT/exp/eoT for one expert over one 512-token half."""
    p, po = e // 2, (e % 2) * 64
    eo = eop.tile([H + 1, 512], F32, name="eo", tag="eo")
    prev = None
    for i in range(KT // 2):
        sc = scp.tile([128, 1024], F32, name="sc", tag="sc")
        for j in range(2):
            kt = 2 * i + j
            nc.tensor.matmul(
                sc[:, j * 512:(j + 1) * 512],
                k_all[po:po + 64, p, kt * 128:(kt + 1) * 128],
                q_all[po:po + 64, p, hsl], start=True, stop=True)
        at = atp.tile([128, 1024], BF16, name="at", tag="at")
        nc.scalar.activation(at, sc, AF.Exp, scale=SCALE)
        if prev is not None:
            pi, pat = prev
            for j in range(2):
                nc.tensor.matmul(eo, v_sb[:, 2 * pi + j, e, :],
                                 pat[:, j * 512:(j + 1) * 512],
                                 start=(pi == 0 and j == 0), stop=False)
        prev = (i, at)
    pi, pat = prev
    for j in range(2):
        nc.tensor.matmul(eo, v_sb[:, 2 * pi + j, e, :],
                         pat[:, j * 512:(j + 1) * 512],
                         start=False, stop=(j == 1))
    return eo


GROUPS = [(0, 3), (3, 5)]  # pair ranges: experts 0-5 and 6-9


def _stage_expert(nc, gat, srowS, eoc, i, e, eo):
    """Drain one expert's eoT psum tile. Engine APs must start at partition
    0/32/64/96, so the sumexp row (psum partition 64, aligned read) goes to a
    partition-0 staging row and then via SBUF->SBUF DMA (unconstrained) into
    row i of the group's batch tile; feature rows go to eoc[:, e].
    """
    se_row = gat.tile([1, 512], F32, name="se_row", tag="se_row", bufs=2)
    nc.vector.tensor_copy(se_row, eo[H:H + 1, :])
    nc.gpsimd.dma_start(out=srowS[i:i + 1, :], in_=se_row)
    nc.vector.tensor_copy(eoc[:, e, :], eo[0:H, :])


def _gate_group(nc, gat, srowS, eoc, gates_g, hsl, dst, p0, np_g):
    """Batched gating for one group of expert pairs on one half: one
    [g,512] reciprocal; scale rows are pulled back to partition 0 via tiny
    SBUF->SBUF DMAs before the broadcast."""
    g = 2 * np_g
    rall = gat.tile([EL, 512], F32, name="rall", tag="rall", bufs=1)
    nc.vector.reciprocal(rall[0:g, :], srowS)
    sall = gat.tile([EL, 512], F32, name="sall", tag="sall", bufs=1)
    nc.vector.tensor_mul(sall[0:g, :], rall[0:g, :], gates_g[:, hsl])
    for i in range(g):
        e = 2 * p0 + i
        scrow = gat.tile([1, 512], F32, name="scrow", tag="scrow",
                         bufs=1)
        nc.gpsimd.dma_start(out=scrow, in_=sall[i:i + 1, :])
        sc64 = gat.tile([H, 512], F32, name="sc64", tag="sc64", bufs=1)
        nc.gpsimd.partition_broadcast(sc64, scrow)
        nc.vector.tensor_mul(dst[(e % 2) * 64:(e % 2) * 64 + 64, e // 2, :],
                             eoc[:, e, :], sc64)


def _emit(tc, xT_d, wq_d, wk_d, bqk_d, wv_d, bv_d, rw_d, rb_d,
          ow_d, ob_d, msel_d, fxo_d, fxi_d, out_d):
    nc = tc.nc
    with (
        tc.tile_pool(name="const", bufs=1) as const,
        tc.tile_pool(name="io", bufs=2) as io,
        tc.tile_pool(name="at", bufs=3) as atp,
        tc.tile_pool(name="gat", bufs=2) as gat,
        tc.tile_pool(name="pj", bufs=2, space="PSUM") as pjp,
        tc.tile_pool(name="sc", bufs=2, space="PSUM") as scp,
        tc.tile_pool(name="eo", bufs=2, space="PSUM") as eop,
    ):
        # ---- small constants via the software-DGE queue ----
        ones_row = const.tile([1, 512], BF16, name="ones_row")
        nc.gpsimd.memset(ones_row, 1.0)
        ones20 = const.tile([E, 1], BF16, name="ones20")
        nc.gpsimd.memset(ones20, 1.0)
        bqk_sb = const.tile([128, 2 * NP_], F32, name="bqk_sb")
        nc.gpsimd.dma_start(out=bqk_sb, in_=bqk_d)
        bv_sb = const.tile([1, EL * H], BF16, name="bv_sb")
        nc.gpsimd.dma_start(out=bv_sb, in_=bv_d)
        rw_sb = const.tile([128, DT, RWX], BF16, name="rw_sb")
        nc.gpsimd.dma_start(
            out=rw_sb, in_=rw_d.rearrange("(t p) e -> p t e", p=128))
        rb_sb = const.tile([E, 1], F32, name="rb_sb")
        nc.gpsimd.dma_start(out=rb_sb, in_=rb_d)
        rbA = const.tile([6, 1], F32, name="rbA")
        nc.gpsimd.dma_start(out=rbA, in_=rb_d[0:6, :])
        rbB = const.tile([4, 1], F32, name="rbB")
        nc.gpsimd.dma_start(out=rbB, in_=rb_d[6:EL, :])
        ob_sb = const.tile([1, D], BF16, name="ob_sb")
        nc.gpsimd.dma_start(out=ob_sb, in_=ob_d)
        msel_sb = const.tile([128, 2], F32, name="msel_sb")
        nc.gpsimd.dma_start(out=msel_sb, in_=msel_d)

        # ---- bulk inputs. The scalar (Activation) queue carries ONLY the
        # odd xT chunks: every DMA instruction on it delays the router /
        # attention Exps queued behind it on the Activation sequencer. The
        # rest rides the SP hardware queue (consumer order: xT, wv, wk, wq,
        # ow) and the gpsimd software queue (late-needed ow half). ----
        xT = const.tile([128, DT, S], BF16, name="xT")
        for dt in range(DT):
            q = nc.sync if dt % 2 == 0 else nc.scalar
            q.dma_start(out=xT[:, dt, :], in_=xT_d[:, dt, :])
        wv_sb = const.tile([128, DT, EL * H], BF16, name="wv_sb")
        wv_in = wv_d.rearrange("(t p) h -> p t h", p=128)
        nc.sync.dma_start(out=wv_sb[:, 0:DT // 2, :],
                          in_=wv_in[:, 0:DT // 2, :])
        nc.sync.dma_start(out=wv_sb[:, DT // 2:, :],
                          in_=wv_in[:, DT // 2:, :])
        wq_sb = const.tile([128, NP_, DT, 128], BF16, name="wq_sb")
        wk_sb = const.tile([128, NP_, DT, 128], BF16, name="wk_sb")
        wq_in = wq_d.rearrange("q (t p) h -> p q t h", p=128)
        wk_in = wk_d.rearrange("q (t p) h -> p q t h", p=128)
        for p in range(NP_):
            nc.sync.dma_start(out=wk_sb[:, p], in_=wk_in[:, p])
            nc.gpsimd.dma_start(out=wq_sb[:, p], in_=wq_in[:, p])
        ow_sb = const.tile([128, DT, D], BF16, name="ow_sb")
        ow_in = ow_d.rearrange("(t p) n -> p t n", p=128)
        nc.sync.dma_start(out=ow_sb[:, 0:DT // 2, :],
                          in_=ow_in[:, 0:DT // 2, :])
        nc.gpsimd.dma_start(out=ow_sb[:, DT // 2:, :],
                            in_=ow_in[:, DT // 2:, :])

        k_all = const.tile([128, NP_, S], BF16, name="k_all")
        q_all = const.tile([128, NP_, S], BF16, name="q_all")
        v_sb = const.tile([128, KT, EL, H + 1], BF16, name="v_sb")
        nc.gpsimd.memset(v_sb[:, :, :, H], 1.0)

        exp_r = const.tile([E, S], BF16, name="exp_r")
        exp_g = [const.tile([6, S], BF16, name="exp_gA"),
                 const.tile([4, S], BF16, name="exp_gB")]
        rs_row = const.tile([1, S], F32, name="rs_row")
        rsum_rec = const.tile([1, S], F32, name="rsum_rec")
        gates_g = [const.tile([6, S], BF16, name="gatesA"),
                   const.tile([4, S], BF16, name="gatesB")]
        rs6 = const.tile([6, S], F32, name="rs6")
        comb = const.tile([128, 2 * NP_, SL], BF16, name="comb")
        combX = const.tile([128, NP_, SL], BF16, name="combX")
        fxs = const.tile([128, 2, NP_ * SL], BF16, name="fxs")
        fx0m = const.tile([128, NP_ * SL], BF16, name="fx0m")

        # ---- router: exp(logits+rb); denominators are folded per pair ----
        # router logits land in one [68,512] psum tile; the host pads
        # rw's columns so the two expert groups repeat at partitions 32 and
        # 64 (engine APs must start at partition 0/32/64/96)
        for ch in range(2):
            sl = slice(ch * 512, (ch + 1) * 512)
            rt = pjp.tile([128, 512], F32, name="rt", tag="pj")
            for dt in range(DT):
                nc.tensor.matmul(rt[0:RWX, :], rw_sb[:, dt, :],
                                 xT[:, dt, sl],
                                 start=(dt == 0), stop=(dt == DT - 1))
            nc.scalar.activation(exp_r[:, sl], rt[0:E, :], AF.Exp, bias=rb_sb)
            nc.scalar.activation(exp_g[0][:, sl], rt[32:38, :], AF.Exp,
                                 bias=rbA)
            nc.scalar.activation(exp_g[1][:, sl], rt[64:68, :], AF.Exp,
                                 bias=rbB)
            rs = pjp.tile([128, 512], F32, name="rs", tag="pj")
            nc.tensor.matmul(rs[0:1, :], ones20, exp_r[:, sl],
                             start=True, stop=True)
            nc.vector.tensor_copy(rs_row[:, sl], rs[0:1, :])
        nc.vector.reciprocal(rsum_rec, rs_row)
        nc.gpsimd.partition_broadcast(rs6, rsum_rec)
        nc.vector.tensor_mul(gates_g[0], exp_g[0], rs6)
        nc.vector.tensor_mul(gates_g[1], exp_g[1], rs6[0:4, :])

        # ---- V projection (natural layout), needed by every expert ----
        for kt in range(KT):
            tsl = slice(kt * 128, (kt + 1) * 128)
            for c2 in range(2):
                csl = slice(c2 * 320, (c2 + 1) * 320)
                vp = pjp.tile([128, 512], F32, name="vp", tag="pj")
                for dt in range(DT):
                    nc.tensor.matmul(vp[:, 0:320], xT[:, dt, tsl],
                                     wv_sb[:, dt, csl],
                                     start=(dt == 0), stop=False)
                nc.tensor.matmul(vp[:, 0:320], ones_row[:, 0:128],
                                 bv_sb[:, csl], start=False, stop=True)
                nc.vector.tensor_copy(
                    v_sb[:, kt, c2 * 5:(c2 + 1) * 5, 0:H],
                    vp[:, 0:320].rearrange("p (e h) -> p e h", e=5))

        # ---- per pair: K/Q projection, then other-half attention (cols
        # 512:1024) for its two experts, gating, eager exchange chunk ----
        oth = slice(512, S)
        for gi, (p0, p1) in enumerate(GROUPS):
            srowS = gat.tile([2 * (p1 - p0), 512], F32, name="srowS",
                             tag="srowS", bufs=2)
            eoc = gat.tile([H, 2 * NP_, 512], BF16, name="eoc", tag="eoc",
                           bufs=1)
            for p in range(p0, p1):
                for (w_sb, dst, bcol) in ((wk_sb, k_all, NP_ + p),
                                          (wq_sb, q_all, p)):
                    for ch in range(2):
                        sl = slice(ch * 512, (ch + 1) * 512)
                        kp = pjp.tile([128, 512], F32, name="kp", tag="pj")
                        for dt in range(DT):
                            nc.tensor.matmul(kp, w_sb[:, p, dt, :],
                                             xT[:, dt, sl], start=(dt == 0),
                                             stop=(dt == DT - 1))
                        nc.vector.tensor_scalar_add(
                            dst[:, p, sl], kp, bqk_sb[:, bcol:bcol + 1])
                for sub in range(2):
                    e = 2 * p + sub
                    eo = _attn_expert(nc, scp, atp, eop, k_all, q_all, v_sb,
                                      e, oth)
                    _stage_expert(nc, gat, srowS, eoc, e - 2 * p0, e, eo)
            _gate_group(nc, gat, srowS, eoc, gates_g[gi], oth, combX,
                        p0, p1 - p0)
            for p in range(p0, p1):
                nc.sync.dma_start(out=fxo_d[:, p], in_=combX[:, p, :])

        # single exchange, emitted after ALL other-half Pool work: the
        # collective parks the gpsimd sequencer for its full duration, so
        # everything queued behind it (own-half gating row-DMAs and
        # broadcasts) stalls — placed here the blockage overlaps the
        # own-half attention's PE/ACT work instead.
        nc.gpsimd.collective_compute(
            "AllGather", mybir.AluOpType.bypass,
            replica_groups=[[0, 1], [2, 3], [4, 5], [6, 7]],
            ins=[fxo_d.opt()], outs=[fxi_d.opt()])
        # NOTE: issue on the SP queue — a scalar-queue DMA would park the
        # Activation sequencer on the collective's semaphore and stall the
        # own-half attention Exps behind it.
        nc.sync.dma_start(
            out=fxs, in_=fxi_d.rearrange("(s p) f -> p s f", p=128))

        # ---- own-half attention (cols 0:512) ----
        own = slice(0, 512)
        for gi, (p0, p1) in enumerate(GROUPS):
            srowS = gat.tile([2 * (p1 - p0), 512], F32, name="srowS",
                             tag="srowS", bufs=2)
            eoc = gat.tile([H, 2 * NP_, 512], BF16, name="eoc", tag="eoc",
                           bufs=1)
            for p in range(p0, p1):
                for sub in range(2):
                    e = 2 * p + sub
                    eo = _attn_expert(nc, scp, atp, eop, k_all, q_all, v_sb,
                                      e, own)
                    _stage_expert(nc, gat, srowS, eoc, e - 2 * p0, e, eo)
            _gate_group(nc, gat, srowS, eoc, gates_g[gi], own, comb,
                        p0, p1 - p0)

        # ---- select partner's slot into comb[:, 5:10] ----
        crem = comb[:, NP_:, :].rearrange("p a b -> p (a b)")
        nc.vector.tensor_scalar_mul(fx0m, fxs[:, 0, :], msel_sb[:, 0:1])
        nc.vector.tensor_scalar_mul(crem, fxs[:, 1, :], msel_sb[:, 1:2])
        nc.vector.tensor_add(crem, crem, fx0m)

        # ---- out projection for own token half ----
        for tt in range(SL // 128):
            tsl = slice(tt * 128, (tt + 1) * 128)
            for (oc, ocw) in OC_CHUNKS:
                osl = slice(oc, oc + ocw)
                op = pjp.tile([128, 512], F32, name="op", tag="pj")
                ht_order = [5, 6, 7, 8, 9, 0, 1, 2, 3, 4]
                for hi, ht in enumerate(ht_order):
                    nc.tensor.matmul(op[:, 0:ocw], comb[:, ht, tsl],
                                     ow_sb[:, ht, osl],
                                     start=(hi == 0), stop=False)
                nc.tensor.matmul(op[:, 0:ocw], ones_row[:, 0:128],
                                 ob_sb[:, osl], start=False, stop=True)
                o_sb = io.tile([128, 512], F32, name="o_sb", tag="o_sb")
                nc.vector.tensor_copy(o_sb[:, 0:ocw], op[:, 0:ocw])
                q = nc.sync if (tt + oc) % 2 == 0 else nc.scalar
                q.dma_start(out=out_d[tsl, osl], in_=o_sb[:, 0:ocw])


def build_nc(loop_n=None):
    nc = bacc.Bacc("TRN2", target_bir_lowering=False, debug=False,
                   num_devices=NCORES)
    xT_d = nc.dram_tensor("xT", [128, DT, S], BF16, kind="ExternalInput").ap()
    wq_d = nc.dram_tensor("wq", [NP_, D, 128], BF16, kind="ExternalInput").ap()
    wk_d = nc.dram_tensor("wk", [NP_, D, 128], BF16, kind="ExternalInput").ap()
    bqk_d = nc.dram_tensor("bqk", [128, 2 * NP_], F32,
                           kind="ExternalInput").ap()
    wv_d = nc.dram_tensor("wv", [D, EL * H], BF16, kind="ExternalInput").ap()
    bv_d = nc.dram_tensor("bv", [1, EL * H], BF16, kind="ExternalInput").ap()
    rw_d = nc.dram_tensor("rw", [D, RWX], BF16, kind="ExternalInput").ap()
    rb_d = nc.dram_tensor("rb", [E, 1], F32, kind="ExternalInput").ap()
    ow_d = nc.dram_tensor("ow", [D, D], BF16, kind="ExternalInput").ap()
    ob_d = nc.dram_tensor("ob", [1, D], BF16, kind="ExternalInput").ap()
    msel_d = nc.dram_tensor("msel", [128, 2], F32, kind="ExternalInput").ap()
    fxo_d = nc.dram_tensor("fx_out", [128, NP_, SL], BF16).ap()
    fxi_d = nc.dram_tensor("fx_in", [256, NP_ * SL], BF16).ap()
    out_d = nc.dram_tensor("out", [SL, D], F32, kind="ExternalOutput").ap()
    with tile.TileContext(nc) as tc:
        if loop_n is None:
            _emit(tc, xT_d, wq_d, wk_d, bqk_d, wv_d, bv_d, rw_d, rb_d,
                  ow_d, ob_d, msel_d, fxo_d, fxi_d, out_d)
        else:
            with tc.For_i(0, loop_n):
                _emit(tc, xT_d, wq_d, wk_d, bqk_d, wv_d, bv_d, rw_d, rb_d,
                      ow_d, ob_d, msel_d, fxo_d, fxi_d, out_d)
    nc.compile()
    return nc


_NC = None


def _get_nc():
    global _NC
    if _NC is None:
        _NC = build_nc()
    return _NC


def _bf(a):
    return np.ascontiguousarray(np.asarray(a, np.float32).astype(NP_BF16))


def make_in_maps(x, wqkv, bqkv, router_w, router_b, out_w, out_b):
    x = np.asarray(x, np.float32)
    wqkv = np.asarray(wqkv, np.float32)
    bqkv = np.asarray(bqkv, np.float32)
    router_w = np.asarray(router_w, np.float32)
    router_b = np.asarray(router_b, np.float32)
    out_w = np.asarray(out_w, np.float32)
    out_b = np.asarray(out_b, np.float32)

    in_maps = []
    for c in range(NCORES):
        b, eh = c // 2, c % 2
        e0 = eh * EL
        own = slice(eh * SL, eh * SL + SL)
        oth = slice((1 - eh) * SL, (1 - eh) * SL + SL)
        eorder = list(range(e0, e0 + EL)) + \
            list(range((1 - eh) * EL, (1 - eh) * EL + EL))

        x_ord = np.concatenate([x[b][own], x[b][oth]], axis=0)  # [1024, D]
        xT = np.ascontiguousarray(
            x_ord.T.reshape(DT, 128, S).transpose(1, 0, 2))

        wq = np.stack([
            np.concatenate([wqkv[2 * (eh * NP_ + p_), :, 0:H],
                            wqkv[2 * (eh * NP_ + p_) + 1, :, 0:H]], axis=1)
            for p_ in range(NP_)])                               # [5, D, 128]
        wk = np.stack([
            np.concatenate([wqkv[2 * (eh * NP_ + p_), :, H:2 * H],
                            wqkv[2 * (eh * NP_ + p_) + 1, :, H:2 * H]],
                           axis=1)
            for p_ in range(NP_)])
        bqk = np.empty((128, 2 * NP_), np.float32)
        for p_ in range(NP_):
            P = eh * NP_ + p_
            bqk[:, p_] = np.concatenate([bqkv[2 * P, 0:H],
                                         bqkv[2 * P + 1, 0:H]])
            bqk[:, NP_ + p_] = np.concatenate([bqkv[2 * P, H:2 * H],
                                               bqkv[2 * P + 1, H:2 * H]])
        wv = np.concatenate(
            [wqkv[e, :, 2 * H:3 * H] for e in range(e0, e0 + EL)], axis=1)
        bv = np.concatenate(
            [bqkv[e, 2 * H:3 * H] for e in range(e0, e0 + EL)])[None, :]
        rw = np.zeros((D, RWX), np.float32)
        rw[:, 0:E] = router_w[:, eorder]
        rw[:, 32:38] = router_w[:, eorder[0:6]]
        rw[:, 64:68] = router_w[:, eorder[6:10]]
        rb = router_b[eorder][:, None].astype(np.float32)
        ow = np.concatenate(
            [out_w[e * H:(e + 1) * H, :] for e in eorder], axis=0)
        msel = np.zeros((128, 2), np.float32)
        msel[:, 0 if eh == 1 else 1] = 1.0

        in_maps.append({
            "xT": _bf(xT), "wq": _bf(wq), "wk": _bf(wk),
            "bqk": np.ascontiguousarray(bqk), "wv": _bf(wv), "bv": _bf(bv),
            "rw": _bf(rw), "rb": np.ascontiguousarray(rb),
            "ow": _bf(ow), "ob": _bf(out_b[None, :]),
            "msel": msel,
        })
    return in_maps


def gather_out(results):
    out = np.empty((B, S, D), np.float32)
    for c in range(NCORES):
        b, eh = c // 2, c % 2
        out[b, eh * SL:(eh + 1) * SL] = results[c]["out"]
    return out


def kernel(x, wqkv, bqkv, router_w, router_b, out_w, out_b):
    nc = _get_nc()
    in_maps = make_in_maps(x, wqkv, bqkv, router_w, router_b, out_w, out_b)
    res = run_bass_kernel_spmd(nc, in_maps, core_ids=list(range(NCORES)))
    return gather_out(res.results)


# revision 5
# speedup vs baseline: 1.0287x; 1.0287x over previous
"""MoE-attention kernel v2 for 8 Trainium2 NeuronCores.

Sharding: expert-parallel within sequence pairs. Cores (2b, 2b+1) both hold
sequence b; core 2b computes experts 0-9, core 2b+1 experts 10-19, each for
ALL 1024 tokens (QKV projections + attention have zero duplicated work; only
the tiny router runs twice). Each core owns the output for its token half;
the 640 feature columns it computed for the partner's half are exchanged via
a pairwise AllGather, then each core runs the full out_proj for its 512
tokens.

Uniform-program tricks (all core-dependence lives in host-prepared data):
  - token order per core: own half first (cols 0-511), other half second
  - expert order per core: own 10 experts first (router cols, wv cols)
  - out_w rows reordered: own experts' feature rows first, partner's second
  - AllGather slot select via per-core 0/1 mask columns

Schedule: V/router first, then per expert pair p: K/Q proj -> other-half
attention for its 2 experts -> gating -> eager fx_out chunk DMA. AllGather
overlaps the own-half attention pass; out_proj runs last. Scores are emitted
one key-tile-pair ahead of the eoT accumulation so the scalar engine's Exp
runs concurrently with PE. Reciprocals are batched per pair ([2,512]).

All matmul operands bf16 (f32 PSUM). Layouts:
  xT       [128, 10dt, 1024t]  host-pre-transposed input
  k/q_all  [128(pair), 5p, 1024t]
  v_sb     [128t, 8kt, 10e, 65]  natural V + ones column -> sumexp row
  scoresT  [128k, 1024q] psum (2 key tiles / psum tile, one Exp per tile)
  eoT      [65h, 512q] accumulated over 8 key tiles
  comb     [128h', 10ht, 512t]  own-half gated features (ht 0-4 local,
           5-9 partner via exchange); combX [128, 5, 512] other-half
  out      [512t, 1280] = comb.T @ ow + ob
"""

import numpy as np

import concourse.bass as bass
import concourse.mybir as mybir
import concourse.tile as tile
from concourse import bacc
from concourse.bass_utils import run_bass_kernel_spmd

F32 = mybir.dt.float32
BF16 = mybir.dt.bfloat16
try:
    import ml_dtypes as _mld
    NP_BF16 = np.dtype(_mld.bfloat16)
except Exception:  # pragma: no cover
    NP_BF16 = np.float32
AF = mybir.ActivationFunctionType

B = 4
S = 1024
D = 1280
E = 20
EL = 10           # experts per core
H = 64
SL = 512          # own-half query tokens per core
DT = D // 128     # 10
KT = S // 128     # 8
NP_ = EL // 2     # 5 own expert pairs
SCALE = float(H) ** -0.5
NCORES = 8
RWX = 68          # router columns incl. aligned group copies
OC_CHUNKS = [(0, 512), (512, 512), (1024, 256)]


def _attn_expert(nc, scp, atp, eop, k_all, q_all, v_sb, e, hsl):
    """Pipelined scoresT/exp/eoT for one expert over one 512-token half."""
    p, po = e // 2, (e % 2) * 64
    eo = eop.tile([H + 1, 512], F32, name="eo", tag="eo")
    prev = None
    for i in range(KT // 2):
        sc = scp.tile([128, 1024], F32, name="sc", tag="sc")
        for j in range(2):
            kt = 2 * i + j
            nc.tensor.matmul(
                sc[:, j * 512:(j + 1) * 512],
                k_all[po:po + 64, p, kt * 128:(kt + 1) * 128],
                q_all[po:po + 64, p, hsl], start=True, stop=True)
        at = atp.tile([128, 1024], BF16, name="at", tag="at")
        nc.scalar.activation(at, sc, AF.Exp, scale=SCALE)
        if prev is not None:
            pi, pat = prev
            for j in range(2):
                nc.tensor.matmul(eo, v_sb[:, 2 * pi + j, e, :],
                                 pat[:, j * 512:(j + 1) * 512],
                                 start=(pi == 0 and j == 0), stop=False)
        prev = (i, at)
    pi, pat = prev
    for j in range(2):
        nc.tensor.matmul(eo, v_sb[:, 2 * pi + j, e, :],
                         pat[:, j * 512:(j + 1) * 512],
                         start=False, stop=(j == 1))
    return eo


GROUPS = [(0, 3), (3, 5)]  # pair ranges: experts 0-5 and 6-9


def _stage_expert(nc, gat, srowS, eoc, i, e, eo):
    """Drain one expert's eoT psum tile. Engine APs must start at partition
    0/32/64/96, so the sumexp row (psum partition 64, aligned read) goes to a
    partition-0 staging row and then via SBUF->SBUF DMA (unconstrained) into
    row i of the group's batch tile; feature rows go to eoc[:, e].
    """
    se_row = gat.tile([1, 512], F32, name="se_row", tag="se_row", bufs=2)
    nc.vector.tensor_copy(se_row, eo[H:H + 1, :])
    nc.gpsimd.dma_start(out=srowS[i:i + 1, :], in_=se_row)
    nc.vector.tensor_copy(eoc[:, e, :], eo[0:H, :])


def _gate_group(nc, gat, srowS, eoc, gates_g, hsl, dst, p0, np_g):
    """Batched gating for one group of expert pairs on one half: one
    [g,512] reciprocal; scale rows are pulled back to partition 0 via tiny
    SBUF->SBUF DMAs before the broadcast."""
    g = 2 * np_g
    rall = gat.tile([EL, 512], F32, name="rall", tag="rall", bufs=1)
    nc.vector.reciprocal(rall[0:g, :], srowS)
    sall = gat.tile([EL, 512], F32, name="sall", tag="sall", bufs=1)
    nc.vector.tensor_mul(sall[0:g, :], rall[0:g, :], gates_g[:, hsl])
    for i in range(g):
        e = 2 * p0 + i
        scrow = gat.tile([1, 512], F32, name="scrow", tag="scrow",
                         bufs=1)
        nc.gpsimd.dma_start(out=scrow, in_=sall[i:i + 1, :])
        sc64 = gat.tile([H, 512], F32, name="sc64", tag="sc64", bufs=1)
        nc.gpsimd.partition_broadcast(sc64, scrow)
        nc.vector.tensor_mul(dst[(e % 2) * 64:(e % 2) * 64 + 64, e // 2, :],
                             eoc[:, e, :], sc64)


def _emit(tc, xT_d, wq_d, wk_d, bqk_d, wv_d, bv_d, rw_d, rb_d,
          ow_d, ob_d, msel_d, fxo_d, fxi_d, out_d):
    nc = tc.nc
    with (
        tc.tile_pool(name="const", bufs=1) as const,
        tc.tile_pool(name="io", bufs=2) as io,
        tc.tile_pool(name="at", bufs=3) as atp,
        tc.tile_pool(name="gat", bufs=2) as gat,
        tc.tile_pool(name="pj", bufs=2, space="PSUM") as pjp,
        tc.tile_pool(name="sc", bufs=2, space="PSUM") as scp,
        tc.tile_pool(name="eo", bufs=2, space="PSUM") as eop,
    ):
        # ---- small constants via the software-DGE queue ----
        ones_row = const.tile([1, 512], BF16, name="ones_row")
        nc.gpsimd.memset(ones_row, 1.0)
        ones20 = const.tile([E, 1], BF16, name="ones20")
        nc.gpsimd.memset(ones20, 1.0)
        bqk_sb = const.tile([128, 2 * NP_], F32, name="bqk_sb")
        nc.gpsimd.dma_start(out=bqk_sb, in_=bqk_d)
        bv_sb = const.tile([1, EL * H], BF16, name="bv_sb")
        nc.gpsimd.dma_start(out=bv_sb, in_=bv_d)
        rw_sb = const.tile([128, DT, RWX], BF16, name="rw_sb")
        nc.gpsimd.dma_start(
            out=rw_sb, in_=rw_d.rearrange("(t p) e -> p t e", p=128))
        rb_sb = const.tile([E, 1], F32, name="rb_sb")
        nc.gpsimd.dma_start(out=rb_sb, in_=rb_d)
        rbA = const.tile([6, 1], F32, name="rbA")
        nc.gpsimd.dma_start(out=rbA, in_=rb_d[0:6, :])
        rbB = const.tile([4, 1], F32, name="rbB")
        nc.gpsimd.dma_start(out=rbB, in_=rb_d[6:EL, :])
        ob_sb = const.tile([1, D], BF16, name="ob_sb")
        nc.gpsimd.dma_start(out=ob_sb, in_=ob_d)
        msel_sb = const.tile([128, 2], F32, name="msel_sb")
        nc.gpsimd.dma_start(out=msel_sb, in_=msel_d)

        # ---- bulk inputs. The scalar (Activation) queue carries ONLY the
        # odd xT chunks: every DMA instruction on it delays the router /
        # attention Exps queued behind it on the Activation sequencer. The
        # rest rides the SP hardware queue (consumer order: xT, wv, wk, wq,
        # ow) and the gpsimd software queue (late-needed ow half). ----
        xT = const.tile([128, DT, S], BF16, name="xT")
        for dt in range(DT):
            q = nc.sync if dt % 2 == 0 else nc.scalar
            q.dma_start(out=xT[:, dt, :], in_=xT_d[:, dt, :])
        wv_sb = const.tile([128, DT, EL * H], BF16, name="wv_sb")
        wv_in = wv_d.rearrange("(t p) h -> p t h", p=128)
        nc.sync.dma_start(out=wv_sb[:, 0:DT // 2, :],
                          in_=wv_in[:, 0:DT // 2, :])
        nc.sync.dma_start(out=wv_sb[:, DT // 2:, :],
                          in_=wv_in[:, DT // 2:, :])
        wq_sb = const.tile([128, NP_, DT, 128], BF16, name="wq_sb")
        wk_sb = const.tile([128, NP_, DT, 128], BF16, name="wk_sb")
        wq_in = wq_d.rearrange("q (t p) h -> p q t h", p=128)
        wk_in = wk_d.rearrange("q (t p) h -> p q t h", p=128)
        for p in range(NP_):
            nc.sync.dma_start(out=wk_sb[:, p], in_=wk_in[:, p])
            nc.gpsimd.dma_start(out=wq_sb[:, p], in_=wq_in[:, p])
        ow_sb = const.tile([128, DT, D], BF16, name="ow_sb")
        ow_in = ow_d.rearrange("(t p) n -> p t n", p=128)
        nc.sync.dma_start(out=ow_sb[:, 0:DT // 2, :],
                          in_=ow_in[:, 0:DT // 2, :])
        nc.gpsimd.dma_start(out=ow_sb[:, DT // 2:, :],
                            in_=ow_in[:, DT // 2:, :])

        k_all = const.tile([128, NP_, S], BF16, name="k_all")
        q_all = const.tile([128, NP_, S], BF16, name="q_all")
        v_sb = const.tile([128, KT, EL, H + 1], BF16, name="v_sb")
        nc.gpsimd.memset(v_sb[:, :, :, H], 1.0)

        exp_r = const.tile([E, S], BF16, name="exp_r")
        exp_g = [const.tile([6, S], BF16, name="exp_gA"),
                 const.tile([4, S], BF16, name="exp_gB")]
        rs_row = const.tile([1, S], F32, name="rs_row")
        rsum_rec = const.tile([1, S], F32, name="rsum_rec")
        gates_g = [const.tile([6, S], BF16, name="gatesA"),
                   const.tile([4, S], BF16, name="gatesB")]
        rs6 = const.tile([6, S], F32, name="rs6")
        comb = const.tile([128, 2 * NP_, SL], BF16, name="comb")
        combX = const.tile([128, NP_, SL], BF16, name="combX")
        fxs = const.tile([128, 2, NP_ * SL], BF16, name="fxs")
        fx0m = const.tile([128, NP_ * SL], BF16, name="fx0m")

        # ---- router: exp(logits+rb); denominators are folded per pair ----
        # router logits land in one [68,512] psum tile; the host pads
        # rw's columns so the two expert groups repeat at partitions 32 and
        # 64 (engine APs must start at partition 0/32/64/96)
        for ch in range(2):
            sl = slice(ch * 512, (ch + 1) * 512)
            rt = pjp.tile([128, 512], F32, name="rt", tag="pj")
            for dt in range(DT):
                nc.tensor.matmul(rt[0:RWX, :], rw_sb[:, dt, :],
                                 xT[:, dt, sl],
                                 start=(dt == 0), stop=(dt == DT - 1))
            nc.scalar.activation(exp_r[:, sl], rt[0:E, :], AF.Exp, bias=rb_sb)
            nc.scalar.activation(exp_g[0][:, sl], rt[32:38, :], AF.Exp,
                                 bias=rbA)
            nc.scalar.activation(exp_g[1][:, sl], rt[64:68, :], AF.Exp,
                                 bias=rbB)
            rs = pjp.tile([128, 512], F32, name="rs", tag="pj")
            nc.tensor.matmul(rs[0:1, :], ones20, exp_r[:, sl],
                             start=True, stop=True)
            nc.vector.tensor_copy(rs_row[:, sl], rs[0:1, :])
        nc.vector.reciprocal(rsum_rec, rs_row)
        nc.gpsimd.partition_broadcast(rs6, rsum_rec)
        nc.vector.tensor_mul(gates_g[0], exp_g[0], rs6)
        nc.vector.tensor_mul(gates_g[1], exp_g[1], rs6[0:4, :])

        # ---- V projection (natural layout), needed by every expert ----
        for kt in range(KT):
            tsl = slice(kt * 128, (kt + 1) * 128)
            for c2 in range(2):
                csl = slice(c2 * 320, (c2 + 1) * 320)
                vp = pjp.tile([128, 512], F32, name="vp", tag="pj")
                for dt in range(DT):
                    nc.tensor.matmul(vp[:, 0:320], xT[:, dt, tsl],
                                     wv_sb[:, dt, csl],
                                     start=(dt == 0), stop=False)
                nc.tensor.matmul(vp[:, 0:320], ones_row[:, 0:128],
                                 bv_sb[:, csl], start=False, stop=True)
                nc.vector.tensor_copy(
                    v_sb[:, kt, c2 * 5:(c2 + 1) * 5, 0:H],
                    vp[:, 0:320].rearrange("p (e h) -> p e h", e=5))

        # ---- per pair: K/Q projection, then other-half attention (cols
        # 512:1024) for its two experts, gating, eager exchange chunk ----
        oth = slice(512, S)
        for gi, (p0, p1) in enumerate(GROUPS):
            srowS = gat.tile([2 * (p1 - p0), 512], F32, name="srowS",
                             tag="srowS", bufs=2)
            eoc = gat.tile([H, 2 * NP_, 512], BF16, name="eoc", tag="eoc",
                           bufs=1)
            for p in range(p0, p1):
                for (w_sb, dst, bcol) in ((wk_sb, k_all, NP_ + p),
                                          (wq_sb, q_all, p)):
                    for ch in range(2):
                        sl = slice(ch * 512, (ch + 1) * 512)
                        kp = pjp.tile([128, 512], F32, name="kp", tag="pj")
                        for dt in range(DT):
                            nc.tensor.matmul(kp, w_sb[:, p, dt, :],
                                             xT[:, dt, sl], start=(dt == 0),
                                             stop=(dt == DT - 1))
                        nc.vector.tensor_scalar_add(
                            dst[:, p, sl], kp, bqk_sb[:, bcol:bcol + 1])
                for sub in range(2):
                    e = 2 * p + sub
                    eo = _attn_expert(nc, scp, atp, eop, k_all, q_all, v_sb,
                                      e, oth)
                    _stage_expert(nc, gat, srowS, eoc, e - 2 * p0, e, eo)
            _gate_group(nc, gat, srowS, eoc, gates_g[gi], oth, combX,
                        p0, p1 - p0)
            for p in range(p0, p1):
                nc.sync.dma_start(out=fxo_d[:, p], in_=combX[:, p, :])

        # single exchange, emitted after ALL other-half Pool work: the
        # collective parks the gpsimd sequencer for its full duration, so
        # everything queued behind it (own-half gating row-DMAs and
        # broadcasts) stalls — placed here the blockage overlaps the
        # own-half attention's PE/ACT work instead.
        nc.gpsimd.collective_compute(
            "AllGather", mybir.AluOpType.bypass,
            replica_groups=[[0, 1], [2, 3], [4, 5], [6, 7]],
            ins=[fxo_d.opt()], outs=[fxi_d.opt()])
        # NOTE: issue on the SP queue — a scalar-queue DMA would park the
        # Activation sequencer on the collective's semaphore and stall the
        # own-half attention Exps behind it.
        nc.sync.dma_start(
            out=fxs, in_=fxi_d.rearrange("(s p) f -> p s f", p=128))

        # ---- own-half attention (cols 0:512). Gating is deferred past
        # the whole pass: the scale chains then run on DVE/Pool while the PE
        # starts out_proj's exchange-ready feature tiles, instead of sitting
        # in the DVE queue between group stagings and stalling eoT reuse ----
        own = slice(0, 512)
        deferred = []
        for gi, (p0, p1) in enumerate(GROUPS):
            srowS = gat.tile([2 * (p1 - p0), 512], F32, name="srowS",
                             tag="srowS", bufs=2)
            eoc = gat.tile([H, 2 * NP_, 512], BF16, name="eoc", tag="eoc",
                           bufs=1)
            for p in range(p0, p1):
                for sub in range(2):
                    e = 2 * p + sub
                    eo = _attn_expert(nc, scp, atp, eop, k_all, q_all, v_sb,
                                      e, own)
                    _stage_expert(nc, gat, srowS, eoc, e - 2 * p0, e, eo)
            deferred.append((srowS, eoc, gi, p0, p1))
        for (srowS, eoc, gi, p0, p1) in deferred:
            _gate_group(nc, gat, srowS, eoc, gates_g[gi], own, comb,
                        p0, p1 - p0)

        # ---- select partner's slot into comb[:, 5:10] ----
        crem = comb[:, NP_:, :].rearrange("p a b -> p (a b)")
        nc.vector.tensor_scalar_mul(fx0m, fxs[:, 0, :], msel_sb[:, 0:1])
        nc.vector.tensor_scalar_mul(crem, fxs[:, 1, :], msel_sb[:, 1:2])
        nc.vector.tensor_add(crem, crem, fx0m)

        # ---- out projection for own token half ----
        for tt in range(SL // 128):
            tsl = slice(tt * 128, (tt + 1) * 128)
            for (oc, ocw) in OC_CHUNKS:
                osl = slice(oc, oc + ocw)
                op = pjp.tile([128, 512], F32, name="op", tag="pj")
                ht_order = [5, 6, 7, 8, 9, 0, 1, 2, 3, 4]
                for hi, ht in enumerate(ht_order):
                    nc.tensor.matmul(op[:, 0:ocw], comb[:, ht, tsl],
                                     ow_sb[:, ht, osl],
                                     start=(hi == 0), stop=False)
                nc.tensor.matmul(op[:, 0:ocw], ones_row[:, 0:128],
                                 ob_sb[:, osl], start=False, stop=True)
                o_sb = io.tile([128, 512], F32, name="o_sb", tag="o_sb")
                nc.vector.tensor_copy(o_sb[:, 0:ocw], op[:, 0:ocw])
                q = nc.sync if (tt + oc) % 2 == 0 else nc.scalar
                q.dma_start(out=out_d[tsl, osl], in_=o_sb[:, 0:ocw])


def build_nc(loop_n=None):
    nc = bacc.Bacc("TRN2", target_bir_lowering=False, debug=False,
                   num_devices=NCORES)
    xT_d = nc.dram_tensor("xT", [128, DT, S], BF16, kind="ExternalInput").ap()
    wq_d = nc.dram_tensor("wq", [NP_, D, 128], BF16, kind="ExternalInput").ap()
    wk_d = nc.dram_tensor("wk", [NP_, D, 128], BF16, kind="ExternalInput").ap()
    bqk_d = nc.dram_tensor("bqk", [128, 2 * NP_], F32,
                           kind="ExternalInput").ap()
    wv_d = nc.dram_tensor("wv", [D, EL * H], BF16, kind="ExternalInput").ap()
    bv_d = nc.dram_tensor("bv", [1, EL * H], BF16, kind="ExternalInput").ap()
    rw_d = nc.dram_tensor("rw", [D, RWX], BF16, kind="ExternalInput").ap()
    rb_d = nc.dram_tensor("rb", [E, 1], F32, kind="ExternalInput").ap()
    ow_d = nc.dram_tensor("ow", [D, D], BF16, kind="ExternalInput").ap()
    ob_d = nc.dram_tensor("ob", [1, D], BF16, kind="ExternalInput").ap()
    msel_d = nc.dram_tensor("msel", [128, 2], F32, kind="ExternalInput").ap()
    fxo_d = nc.dram_tensor("fx_out", [128, NP_, SL], BF16).ap()
    fxi_d = nc.dram_tensor("fx_in", [256, NP_ * SL], BF16).ap()
    out_d = nc.dram_tensor("out", [SL, D], F32, kind="ExternalOutput").ap()
    with tile.TileContext(nc) as tc:
        if loop_n is None:
            _emit(tc, xT_d, wq_d, wk_d, bqk_d, wv_d, bv_d, rw_d, rb_d,
                  ow_d, ob_d, msel_d, fxo_d, fxi_d, out_d)
        else:
            with tc.For_i(0, loop_n):
                _emit(tc, xT_d, wq_d, wk_d, bqk_d, wv_d, bv_d, rw_d, rb_d,
                      ow_d, ob_d, msel_d, fxo_d, fxi_d, out_d)
    nc.compile()
    return nc


_NC = None


def _get_nc():
    global _NC
    if _NC is None:
        _NC = build_nc()
    return _NC


def _bf(a):
    return np.ascontiguousarray(np.asarray(a, np.float32).astype(NP_BF16))


def make_in_maps(x, wqkv, bqkv, router_w, router_b, out_w, out_b):
    x = np.asarray(x, np.float32)
    wqkv = np.asarray(wqkv, np.float32)
    bqkv = np.asarray(bqkv, np.float32)
    router_w = np.asarray(router_w, np.float32)
    router_b = np.asarray(router_b, np.float32)
    out_w = np.asarray(out_w, np.float32)
    out_b = np.asarray(out_b, np.float32)

    in_maps = []
    for c in range(NCORES):
        b, eh = c // 2, c % 2
        e0 = eh * EL
        own = slice(eh * SL, eh * SL + SL)
        oth = slice((1 - eh) * SL, (1 - eh) * SL + SL)
        eorder = list(range(e0, e0 + EL)) + \
            list(range((1 - eh) * EL, (1 - eh) * EL + EL))

        x_ord = np.concatenate([x[b][own], x[b][oth]], axis=0)  # [1024, D]
        xT = np.ascontiguousarray(
            x_ord.T.reshape(DT, 128, S).transpose(1, 0, 2))

        wq = np.stack([
            np.concatenate([wqkv[2 * (eh * NP_ + p_), :, 0:H],
                            wqkv[2 * (eh * NP_ + p_) + 1, :, 0:H]], axis=1)
            for p_ in range(NP_)])                               # [5, D, 128]
        wk = np.stack([
            np.concatenate([wqkv[2 * (eh * NP_ + p_), :, H:2 * H],
                            wqkv[2 * (eh * NP_ + p_) + 1, :, H:2 * H]],
                           axis=1)
            for p_ in range(NP_)])
        bqk = np.empty((128, 2 * NP_), np.float32)
        for p_ in range(NP_):
            P = eh * NP_ + p_
            bqk[:, p_] = np.concatenate([bqkv[2 * P, 0:H],
                                         bqkv[2 * P + 1, 0:H]])
            bqk[:, NP_ + p_] = np.concatenate([bqkv[2 * P, H:2 * H],
                                               bqkv[2 * P + 1, H:2 * H]])
        wv = np.concatenate(
            [wqkv[e, :, 2 * H:3 * H] for e in range(e0, e0 + EL)], axis=1)
        bv = np.concatenate(
            [bqkv[e, 2 * H:3 * H] for e in range(e0, e0 + EL)])[None, :]
        rw = np.zeros((D, RWX), np.float32)
        rw[:, 0:E] = router_w[:, eorder]
        rw[:, 32:38] = router_w[:, eorder[0:6]]
        rw[:, 64:68] = router_w[:, eorder[6:10]]
        rb = router_b[eorder][:, None].astype(np.float32)
        ow = np.concatenate(
            [out_w[e * H:(e + 1) * H, :] for e in eorder], axis=0)
        msel = np.zeros((128, 2), np.float32)
        msel[:, 0 if eh == 1 else 1] = 1.0

        in_maps.append({
            "xT": _bf(xT), "wq": _bf(wq), "wk": _bf(wk),
            "bqk": np.ascontiguousarray(bqk), "wv": _bf(wv), "bv": _bf(bv),
            "rw": _bf(rw), "rb": np.ascontiguousarray(rb),
            "ow": _bf(ow), "ob": _bf(out_b[None, :]),
            "msel": msel,
        })
    return in_maps


def gather_out(results):
    out = np.empty((B, S, D), np.float32)
    for c in range(NCORES):
        b, eh = c // 2, c % 2
        out[b, eh * SL:(eh + 1) * SL] = results[c]["out"]
    return out


def kernel(x, wqkv, bqkv, router_w, router_b, out_w, out_b):
    nc = _get_nc()
    in_maps = make_in_maps(x, wqkv, bqkv, router_w, router_b, out_w, out_b)
    res = run_bass_kernel_spmd(nc, in_maps, core_ids=list(range(NCORES)))
    return gather_out(res.results)
